# revision 12
# baseline (speedup 1.0000x reference)
"""Trainium2 Bass kernel for nn_DihedralBiasVMap2 (dihedral-bias ensemble MLP).

Sharding: ensemble (model) axis of the MLP weights is split across 8 cores
(4 models each). Dihedral geometry + forward/backward of the local model
shard run on-device; per-model energy sums and jac statistics (sum, sum of
squares over models) are AllReduced on-device; every core then computes the
variance switch sigma and the sigma-scaled per-CV force contributions.
Host side does only data layout (gather of the 16k indexed atom rows,
padding/transpose of weights) and the final scatter-add of per-CV force
contributions into the full [1e6,3] forces array.

Compute layout notes:
- CV tiles are [128, T] with CV index c = t*128 + p (partition p, column t).
- The MLP runs on a single x vector per model; all matvecs use PE
  "orientation A": stationary lhsT = weight tile [K=128, M=128], moving
  rhs = activation column [128, 1], so activations stay in column layout
  [128, n_chunks] with zero transposes anywhere (backward uses pre-
  transposed weight copies uploaded from host).
- cos/sin of the torsion are computed algebraically (cos = x/r, sin = y/r
  from the dihedral dot/cross products) - no trig tables needed.
"""

import os
from contextlib import ExitStack
from dataclasses import dataclass

import numpy as np
import ml_dtypes

import concourse.bass as bass
import concourse.bacc as bacc
import concourse.mybir as mybir
import concourse.tile as tile
from concourse.bass_utils import run_bass_kernel_spmd

F32 = mybir.dt.float32
MUL = mybir.AluOpType.mult
ADD = mybir.AluOpType.add
SUB = mybir.AluOpType.subtract
ISGT = mybir.AluOpType.is_gt
ISGE = mybir.AluOpType.is_ge
ISLT = mybir.AluOpType.is_lt
MAXOP = mybir.AluOpType.max
MINOP = mybir.AluOpType.min

MAGIC = 12582912.0  # 1.5 * 2**23: float32 round-to-nearest-integer constant


@dataclass(frozen=True)
class Cfg:
    n_cores: int = 8
    nm: int = 4            # models per core
    t_cols: int = 32       # C_pad / 128
    oc: int = 4            # hidden / 128
    c_true: int = 4000
    e0: float = 2.0
    e1: float = 3.0
    cdt: str = "f32"       # "f32" or "bf16"

    @property
    def c_pad(self):
        return 128 * self.t_cols

    @property
    def ic1(self):
        return 2 * self.t_cols   # input chunks for layer 1 (IN = 2*c_pad)

    @property
    def in_dim(self):
        return 128 * self.ic1

    @property
    def hid(self):
        return 128 * self.oc

    @property
    def m_total(self):
        return self.n_cores * self.nm

    @property
    def mdt(self):
        return F32 if self.cdt == "f32" else mybir.dt.bfloat16

    @property
    def npdt(self):
        return np.float32 if self.cdt == "f32" else ml_dtypes.bfloat16


REAL_CFG = Cfg()


def _build_nc(cfg: Cfg):
    # Bacc (not raw Bass): its finalize runs the TRN2 lowering passes that
    # split multi-semaphore waits (HW allows one wait per instruction).
    nc = bacc.Bacc("TRN2")
    f32 = F32
    cdt = cfg.mdt
    T = cfg.t_cols
    OC = cfg.oc
    IC1 = cfg.ic1
    NM = cfg.nm
    HID = cfg.hid

    # ------------------- DRAM I/O -------------------
    geom_d = nc.dram_tensor("geom", [128, 4, 3, T], f32, kind="ExternalInput")
    boxq_d = nc.dram_tensor("boxq", [128, 2, 3, T], f32, kind="ExternalInput")
    maskdiv_d = nc.dram_tensor("maskdiv", [128, T], f32, kind="ExternalInput")
    w1_d = nc.dram_tensor("w1", [NM, cfg.in_dim, HID], cdt, kind="ExternalInput")
    w1t_d = nc.dram_tensor("w1t", [NM, HID, cfg.in_dim], cdt, kind="ExternalInput")
    wh_d = nc.dram_tensor("wh", [3, NM, HID, HID], cdt, kind="ExternalInput")
    wht_d = nc.dram_tensor("wht", [3, NM, HID, HID], cdt, kind="ExternalInput")
    w5c_d = nc.dram_tensor("w5c", [128, NM * OC], cdt, kind="ExternalInput")
    w5f_d = nc.dram_tensor("w5f", [128, NM * OC], f32, kind="ExternalInput")
    bias_d = nc.dram_tensor("bias", [128, NM, 4, OC], f32, kind="ExternalInput")
    b5v_d = nc.dram_tensor("b5v", [1, NM], f32, kind="ExternalInput")

    energy_d = nc.dram_tensor("energy", [1, 1], f32, kind="ExternalOutput")
    fcon_d = nc.dram_tensor("fcontrib", [128, 4, 3, T], f32, kind="ExternalOutput")

    with ExitStack() as ctx:
        tc = ctx.enter_context(tile.TileContext(nc))
        geo = ctx.enter_context(tc.tile_pool(name="geo", bufs=1))
        gtmp = ctx.enter_context(tc.tile_pool(name="gtmp", bufs=2))
        w1pool = ctx.enter_context(tc.tile_pool(name="w1pool", bufs=3))
        w1tpool = ctx.enter_context(tc.tile_pool(name="w1tpool", bufs=2))
        whpool = ctx.enter_context(tc.tile_pool(name="whpool", bufs=4))
        hpool = ctx.enter_context(tc.tile_pool(name="hpool", bufs=2))
        zpool = ctx.enter_context(tc.tile_pool(name="zpool", bufs=1, space="PSUM"))
        gxpool = ctx.enter_context(tc.tile_pool(name="gxpool", bufs=1, space="PSUM"))
        epool = ctx.enter_context(tc.tile_pool(name="epool", bufs=1, space="PSUM"))
        dram = ctx.enter_context(tc.tile_pool(name="dram", bufs=1, space="DRAM"))

        def vt(out, a, b, op):
            nc.vector.tensor_tensor(out=out, in0=a, in1=b, op=op)

        def vs(out, a, s1, s2=None, op0=MUL, op1=ADD):
            if s2 is None:
                nc.vector.tensor_scalar(out, a, s1, None, op0)
            else:
                nc.vector.tensor_scalar(out, a, s1, s2, op0, op1)

        def gtile(shape, name, dt=f32):
            return geo.tile(shape, dt, name=name, tag=name)

        def dot3(a, b, name):
            out = gtile([128, T], name)
            tmp = gtile([128, T], name + "_dt")
            vt(out, a[:, 0, :], b[:, 0, :], MUL)
            vt(tmp, a[:, 1, :], b[:, 1, :], MUL)
            vt(out, out, tmp, ADD)
            vt(tmp, a[:, 2, :], b[:, 2, :], MUL)
            vt(out, out, tmp, ADD)
            return out

        def cross3(a, b, name):
            out = gtile([128, 3, T], name)
            tmp = gtile([128, T], name + "_ct")
            for i, (i1, i2) in enumerate([(1, 2), (2, 0), (0, 1)]):
                vt(out[:, i, :], a[:, i1, :], b[:, i2, :], MUL)
                vt(tmp, a[:, i2, :], b[:, i1, :], MUL)
                vt(out[:, i, :], out[:, i, :], tmp, SUB)
            return out

        def refined_sqrt(s, name):
            """sqrt with one Newton step: y' = 0.5*(y + s/y)."""
            y = gtile(list(s.shape), name)
            nc.scalar.activation(y, s, mybir.ActivationFunctionType.Sqrt)
            iy = gtile(list(s.shape), name + "_i")
            nc.vector.reciprocal(iy, y)
            t = gtile(list(s.shape), name + "_n")
            vt(t, s, iy, MUL)
            vt(t, t, y, ADD)
            vs(t, t, 0.5)
            return t

        # ------------------- geometry -------------------
        g_sb = geo.tile([128, 4, 3, T], f32, name="g_sb", tag="g_sb")
        nc.sync.dma_start(out=g_sb, in_=geom_d[:, :, :, :])
        boxq_sb = geo.tile([128, 2, 3, T], f32, name="boxq_sb", tag="boxq_sb")
        nc.sync.dma_start(out=boxq_sb, in_=boxq_d[:, :, :, :])
        maskdiv_sb = geo.tile([128, T], f32, name="maskdiv_sb", tag="maskdiv_sb")
        nc.sync.dma_start(out=maskdiv_sb, in_=maskdiv_d[:, :])

        wp = []
        for a in range(4):
            q = gtile([128, 3, T], f"q{a}")
            vt(q, g_sb[:, a], boxq_sb[:, 1], MUL)           # p * invbox
            r = gtile([128, 3, T], f"r{a}")
            vs(r, q, MAGIC, -MAGIC, ADD, ADD)               # round-to-int
            gt = gtile([128, 3, T], f"gt{a}")
            vt(gt, r, q, ISGT)
            vt(r, r, gt, SUB)                               # floor(q)
            vt(r, r, boxq_sb[:, 0], MUL)                    # floor*box
            w = gtile([128, 3, T], f"wp{a}")
            vt(w, g_sb[:, a], r, SUB)                       # wrapped coords
            wp.append(w)

        b1 = gtile([128, 3, T], "b1")
        b2 = gtile([128, 3, T], "b2")
        b3 = gtile([128, 3, T], "b3")
        vt(b1, wp[1], wp[0], SUB)
        vt(b2, wp[2], wp[1], SUB)
        vt(b3, wp[3], wp[2], SUB)

        n1 = cross3(b1, b2, "n1")
        n2 = cross3(b2, b3, "n2")
        tnn = cross3(n2, n1, "tnn")

        xd = dot3(n1, n2, "xd")          # = |n1||n2| cos(phi)
        yd0 = dot3(tnn, b2, "yd0")       # = |n1||n2||b2| sin(phi)
        s2d = dot3(b2, b2, "s2d")
        n1sq = dot3(n1, n1, "n1sq")
        n2sq = dot3(n2, n2, "n2sq")
        d12 = dot3(b1, b2, "d12")
        d32 = dot3(b3, b2, "d32")

        nb2 = refined_sqrt(s2d, "nb2")
        inb2 = gtile([128, T], "inb2")
        nc.vector.reciprocal(inb2, nb2)
        ib2 = gtile([128, T], "ib2")
        nc.vector.reciprocal(ib2, s2d)
        yd = gtile([128, T], "yd")
        vt(yd, yd0, inb2, MUL)

        r2 = gtile([128, T], "r2")
        tmpg = gtile([128, T], "tmpg")
        vt(r2, xd, xd, MUL)
        vt(tmpg, yd, yd, MUL)
        vt(r2, r2, tmpg, ADD)
        rr = refined_sqrt(r2, "rr")
        ir = gtile([128, T], "ir")
        nc.vector.reciprocal(ir, rr)
        cos_t = gtile([128, T], "cos_t")
        sin_t = gtile([128, T], "sin_t")
        vt(cos_t, xd, ir, MUL)
        vt(sin_t, yd, ir, MUL)

        i1 = gtile([128, T], "i1")
        i2 = gtile([128, T], "i2")
        nc.vector.reciprocal(i1, n1sq)
        nc.vector.reciprocal(i2, n2sq)
        A = gtile([128, T], "Acf")
        B = gtile([128, T], "Bcf")
        vt(A, d12, ib2, MUL)
        vt(B, d32, ib2, MUL)
        cF = gtile([128, T], "cF")
        cH = gtile([128, T], "cH")
        vt(cF, nb2, i1, MUL)
        vt(cH, nb2, i2, MUL)
        Fc = gtile([128, 3, T], "Fc")
        Hc = gtile([128, 3, T], "Hc")
        for c in range(3):
            vt(Fc[:, c, :], n1[:, c, :], cF, MUL)
            vt(Hc[:, c, :], n2[:, c, :], cH, MUL)
        cA = gtile([128, T], "cA")      # -(1+A)
        vs(cA, A, -1.0, -1.0, MUL, ADD)
        cB = gtile([128, T], "cB")      # 1+B
        vs(cB, B, 1.0, None, ADD)
        g1 = gtile([128, 3, T], "g1")
        g2 = gtile([128, 3, T], "g2")
        t1g = gtile([128, T], "t1g")
        for c in range(3):
            vt(g1[:, c, :], Fc[:, c, :], cA, MUL)
            vt(t1g, Hc[:, c, :], B, MUL)
            vt(g1[:, c, :], g1[:, c, :], t1g, SUB)
            vt(g2[:, c, :], Fc[:, c, :], A, MUL)
            vt(t1g, Hc[:, c, :], cB, MUL)
            vt(g2[:, c, :], g2[:, c, :], t1g, ADD)

        # x vector in compute dtype, column layout [128, 2T]
        x_sb = geo.tile([128, IC1], cdt, name="x_sb", tag="x_sb")
        nc.vector.tensor_copy(out=x_sb[:, 0:T], in_=cos_t)
        nc.vector.tensor_copy(out=x_sb[:, T:2 * T], in_=sin_t)

        # ------------------- small params -------------------
        w5c_sb = geo.tile([128, NM * OC], cdt, name="w5c_sb", tag="w5c_sb")
        nc.sync.dma_start(out=w5c_sb, in_=w5c_d[:, :])
        w5f_sb = geo.tile([128, NM * OC], f32, name="w5f_sb", tag="w5f_sb")
        nc.sync.dma_start(out=w5f_sb, in_=w5f_d[:, :])
        bias_sb = geo.tile([128, NM, 4, OC], f32, name="bias_sb", tag="bias_sb")
        nc.sync.dma_start(out=bias_sb, in_=bias_d[:, :, :, :])
        b5v_sb = geo.tile([1, NM], f32, name="b5v_sb", tag="b5v_sb")
        nc.sync.dma_start(out=b5v_sb, in_=b5v_d[:, :])

        s1 = geo.tile([128, T], f32, name="s1", tag="s1")
        s2t = geo.tile([128, T], f32, name="s2t", tag="s2t")
        nc.vector.memset(s1, 0.0)
        nc.vector.memset(s2t, 0.0)

        pe_ps = epool.tile([1, NM], f32, name="pe_ps", tag="pe_ps")

        # ------------------- per-model MLP fwd+bwd -------------------
        GJ = min(8, IC1)  # layer-1 i-chunks per DMA
        for m in range(NM):
            w1r = w1_d[m].rearrange("(c p) o -> p c o", p=128)
            # one single-column psum tile per output chunk: each lives in its
            # own PSUM bank so the column accumulation groups may interleave
            pz1s = [zpool.tile([128, 1], f32, name=f"pz1c{o}", tag=f"pz1c{o}")
                    for o in range(OC)]
            for j in range(IC1 // GJ):
                w1sb = w1pool.tile([128, GJ, HID], cdt, name="w1sb", tag="w1sb")
                nc.sync.dma_start(out=w1sb, in_=w1r[:, j * GJ:(j + 1) * GJ, :])
                for c in range(GJ):
                    jj = j * GJ + c
                    for o in range(OC):
                        nc.tensor.matmul(
                            pz1s[o][:, 0:1],
                            w1sb[:, c, o * 128:(o + 1) * 128],
                            x_sb[:, jj:jj + 1],
                            start=(jj == 0), stop=(jj == IC1 - 1),
                        )
            zf = hpool.tile([128, OC], f32, name="zf", tag="zf")
            for o in range(OC):
                vt(zf[:, o:o + 1], pz1s[o][:, 0:1], bias_sb[:, m, 0, o:o + 1], ADD)
            h = hpool.tile([128, OC], cdt, name="h1", tag="h1")
            nc.vector.tensor_relu(h, zf)
            hs = [h]
            for l in range(3):
                whsb = whpool.tile([128, OC, HID], cdt, name="whsb", tag="whsb")
                nc.sync.dma_start(
                    out=whsb, in_=wh_d[l, m].rearrange("(c p) o -> p c o", p=128))
                pz = zpool.tile([128, OC], f32, name="pz", tag="pz")
                for o in range(OC):
                    for ic in range(OC):
                        nc.tensor.matmul(
                            pz[:, o:o + 1],
                            whsb[:, ic, o * 128:(o + 1) * 128],
                            hs[-1][:, ic:ic + 1],
                            start=(ic == 0), stop=(ic == OC - 1),
                        )
                zf = hpool.tile([128, OC], f32, name="zf", tag="zf")
                vt(zf, pz, bias_sb[:, m, l + 1, :], ADD)
                h = hpool.tile([128, OC], cdt, name=f"h{l + 2}", tag=f"h{l + 2}")
                nc.vector.tensor_relu(h, zf)
                hs.append(h)
            # energy: e_m = h4 . W5[m]
            for j in range(OC):
                nc.tensor.matmul(
                    pe_ps[0:1, m:m + 1],
                    w5c_sb[:, m * OC + j:m * OC + j + 1],
                    hs[3][:, j:j + 1],
                    start=(j == 0), stop=(j == OC - 1),
                )
            # backward
            msk = hpool.tile([128, OC], f32, name="msk", tag="msk")
            nc.vector.tensor_scalar(msk, hs[3], 0.0, None, ISGT)
            gz = hpool.tile([128, OC], cdt, name="gz", tag="gz")
            vt(gz, w5f_sb[:, m * OC:(m + 1) * OC], msk, MUL)
            for l in [2, 1, 0]:
                whtsb = whpool.tile([128, OC, HID], cdt, name="whtsb", tag="whsb")
                nc.sync.dma_start(
                    out=whtsb, in_=wht_d[l, m].rearrange("(c p) i -> p c i", p=128))
                pg = zpool.tile([128, OC], f32, name="pz", tag="pz")
                for ic in range(OC):
                    for o in range(OC):
                        nc.tensor.matmul(
                            pg[:, ic:ic + 1],
                            whtsb[:, o, ic * 128:(ic + 1) * 128],
                            gz[:, o:o + 1],
                            start=(o == 0), stop=(o == OC - 1),
                        )
                msk = hpool.tile([128, OC], f32, name="msk", tag="msk")
                nc.vector.tensor_scalar(msk, hs[l], 0.0, None, ISGT)
                gz = hpool.tile([128, OC], cdt, name="gz", tag="gz")
                vt(gz, pg, msk, MUL)
            # layer-1 backward -> gx in column layout [128, IC1].
            # All OC o-chunk weight tiles for an i-block are resident at once
            # so each gx column's accumulation group runs to completion before
            # the next column starts (one pending group per PSUM bank).
            pgx = gxpool.tile([128, IC1], f32, name="pgx", tag="pgx")
            w1tr = w1t_d[m].rearrange("(c p) i -> p c i", p=128)
            IB = min(16, IC1)  # i-chunks per block
            for ib in range(IC1 // IB):
                w1tsbs = []
                for o in range(OC):
                    w1tsb = w1tpool.tile([128, IB * 128], cdt,
                                         name=f"w1tsb{o}", tag=f"w1tsb{o}")
                    nc.sync.dma_start(
                        out=w1tsb,
                        in_=w1tr[:, o, ib * IB * 128:(ib + 1) * IB * 128])
                    w1tsbs.append(w1tsb)
                for ci in range(IB):
                    ic = ib * IB + ci
                    for o in range(OC):
                        nc.tensor.matmul(
                            pgx[:, ic:ic + 1],
                            w1tsbs[o][:, ci * 128:(ci + 1) * 128],
                            gz[:, o:o + 1],
                            start=(o == 0), stop=(o == OC - 1),
                        )
            # jac_m = cos .* gx_hi - sin .* gx_lo ; accumulate stats
            jac = gtmp.tile([128, T], f32, name="jac", tag="jac")
            jtmp = gtmp.tile([128, T], f32, name="jtmp", tag="jtmp")
            vt(jac, cos_t, pgx[:, T:2 * T], MUL)
            vt(jtmp, sin_t, pgx[:, 0:T], MUL)
            vt(jac, jac, jtmp, SUB)
            vt(s1, s1, jac, ADD)
            vt(jtmp, jac, jac, MUL)
            vt(s2t, s2t, jtmp, ADD)

        # ------------------- cross-core reduction -------------------
        el = geo.tile([1, NM], f32, name="el", tag="el")
        vt(el, pe_ps[0:1, :], b5v_sb, ADD)
        ecol = geo.tile([128, 1], f32, name="ecol", tag="ecol")
        nc.vector.memset(ecol, 0.0)
        nc.vector.tensor_reduce(ecol[0:1, 0:1], el, mybir.AxisListType.X, ADD)

        ccin = dram.tile([128, 2 * T + 1], f32, name="ccin")
        cc_addr = "Shared" if cfg.n_cores > 4 else "Local"
        ccout = dram.tile([128, 2 * T + 1], f32, name="ccout", addr_space=cc_addr)
        nc.sync.dma_start(out=ccin[:, 0:T], in_=s1)
        nc.sync.dma_start(out=ccin[:, T:2 * T], in_=s2t)
        nc.sync.dma_start(out=ccin[:, 2 * T:2 * T + 1], in_=ecol)
        nc.gpsimd.collective_compute(
            "AllReduce",
            ADD,
            replica_groups=[list(range(cfg.n_cores))],
            ins=[ccin.opt()],
            outs=[ccout.opt()],
        )
        cc_sb = geo.tile([128, 2 * T + 1], f32, name="cc_sb", tag="cc_sb")
        nc.sync.dma_start(out=cc_sb, in_=ccout[:, :])

        # ------------------- sigma switch + outputs -------------------
        M = float(cfg.m_total)
        gbar = geo.tile([128, T], f32, name="gbar", tag="gbar")
        vs(gbar, cc_sb[:, 0:T], 1.0 / M)
        vvar = geo.tile([128, T], f32, name="vvar", tag="vvar")
        vt(vvar, gbar, gbar, MUL)
        nc.vector.scalar_tensor_tensor(
            vvar, vvar, -M, cc_sb[:, T:2 * T], MUL, ADD)   # S2 - M*gbar^2
        vt(vvar, vvar, maskdiv_sb, MUL)                    # mask / (M-1)
        rowsum = geo.tile([128, 1], f32, name="rowsum", tag="rowsum")
        nc.vector.tensor_reduce(rowsum, vvar, mybir.AxisListType.X, ADD)
        ones_t = geo.tile([128, 1], f32, name="ones_t", tag="ones_t")
        nc.vector.memset(ones_t, 1.0)
        tot_ps = epool.tile([1, 1], f32, name="tot_ps", tag="tot_ps")
        nc.tensor.matmul(tot_ps[0:1, 0:1], ones_t, rowsum, start=True, stop=True)
        md = geo.tile([1, 1], f32, name="md", tag="md")
        nc.scalar.activation(md, tot_ps, mybir.ActivationFunctionType.Sqrt,
                             scale=1.0 / cfg.c_true)
        # iswitch and smooth switch
        den = cfg.e1 - cfg.e0
        isw = geo.tile([1, 1], f32, name="isw", tag="isw")
        vs(isw, md, -1.0 / den, cfg.e1 / den, MUL, ADD)
        arg = geo.tile([1, 1], f32, name="arg", tag="arg")
        pi = float(np.pi)
        vs(arg, md, pi / den, pi / 2 - pi * cfg.e1 / den, MUL, ADD)
        vs(arg, arg, -pi / 2, pi / 2, MAXOP, MINOP)
        sinv = geo.tile([1, 1], f32, name="sinv", tag="sinv")
        nc.scalar.activation(sinv, arg, mybir.ActivationFunctionType.Sin)
        smooth = geo.tile([1, 1], f32, name="smooth", tag="smooth")
        vs(smooth, sinv, -0.5, 0.5, MUL, ADD)
        ge1 = geo.tile([1, 1], f32, name="ge1", tag="ge1")
        nc.vector.tensor_scalar(ge1, isw, 1.0, None, ISGE)
        lt0 = geo.tile([1, 1], f32, name="lt0", tag="lt0")
        nc.vector.tensor_scalar(lt0, isw, 0.0, None, ISLT)
        na = geo.tile([1, 1], f32, name="na", tag="na")
        vs(na, ge1, -1.0, 1.0, MUL, ADD)
        nb = geo.tile([1, 1], f32, name="nb", tag="nb")
        vs(nb, lt0, -1.0, 1.0, MUL, ADD)
        sig = geo.tile([1, 1], f32, name="sig", tag="sig")
        vt(sig, na, nb, MUL)
        vt(sig, sig, smooth, MUL)
        vt(sig, sig, ge1, ADD)

        eout = geo.tile([1, 1], f32, name="eout", tag="eout")
        vs(eout, cc_sb[0:1, 2 * T:2 * T + 1], 1.0 / M)
        vt(eout, eout, sig, MUL)
        nc.sync.dma_start(out=energy_d[:, :], in_=eout)

        # broadcast sigma across partitions via DRAM bounce
        sigd = dram.tile([1, 1], f32, name="sigd")
        nc.sync.dma_start(out=sigd[:, :], in_=sig)
        sigb = geo.tile([128, 1], f32, name="sigb", tag="sigb")
        nc.sync.dma_start(out=sigb, in_=sigd.to_broadcast((128, 1)))

        wv = geo.tile([128, T], f32, name="wv", tag="wv")
        nc.vector.tensor_scalar(wv, gbar, sigb, None, MUL)
        wneg = geo.tile([128, T], f32, name="wneg", tag="wneg")
        vs(wneg, wv, -1.0)
        fcon = geo.tile([128, 4, 3, T], f32, name="fcon", tag="fcon")
        for c in range(3):
            vt(fcon[:, 0, c, :], Fc[:, c, :], wv, MUL)
            vt(fcon[:, 1, c, :], g1[:, c, :], wv, MUL)
            vt(fcon[:, 2, c, :], g2[:, c, :], wv, MUL)
            vt(fcon[:, 3, c, :], Hc[:, c, :], wneg, MUL)
        nc.sync.dma_start(out=fcon_d[:, :, :, :], in_=fcon)

    nc.finalize()   # Bacc: runs lowering passes (reg alloc, wait splitting)
    return nc


# ---------------------------------------------------------------------------
# host-side data prep
# ---------------------------------------------------------------------------

def _make_in_maps(inputs, cfg: Cfg):
    npdt = cfg.npdt
    T = cfg.t_cols
    NM = cfg.nm
    OC = cfg.oc
    HID = cfg.hid
    C_PAD = cfg.c_pad
    C_TRUE = cfg.c_true
    IN = cfg.in_dim
    CIN_TRUE = 2 * C_TRUE

    positions = np.asarray(inputs["positions"], np.float32)
    colvar = np.asarray(inputs["colvar_idx"], np.int32)
    box = np.diag(np.asarray(inputs["boxvectors"], np.float32)).copy()

    # padded CV index table (repeat row 0; padded rows are masked out)
    pad = np.zeros((C_PAD, 4), np.int32)
    pad[:C_TRUE] = colvar
    pad[C_TRUE:] = colvar[0]
    flat = pad.reshape(-1)
    sel = positions[flat].reshape(C_PAD, 4, 3)           # host gather
    geom = np.ascontiguousarray(
        sel.reshape(T, 128, 4, 3).transpose(1, 2, 3, 0)).astype(np.float32)

    boxq = np.empty((128, 2, 3, T), np.float32)
    boxq[:, 0] = box[None, :, None]
    boxq[:, 1] = (np.float32(1.0) / box)[None, :, None]

    cv_idx = (np.arange(T)[None, :] * 128 + np.arange(128)[:, None])
    maskdiv = ((cv_idx < C_TRUE).astype(np.float32)
               / np.float32(cfg.m_total - 1))

    W1 = np.asarray(inputs["W1"], np.float32)
    Whs = [np.asarray(inputs[f"W{i}"], np.float32) for i in (2, 3, 4)]
    W5 = np.asarray(inputs["W5"], np.float32)
    bs = [np.asarray(inputs[f"b{i}"], np.float32) for i in (1, 2, 3, 4)]
    b5 = np.asarray(inputs["b5"], np.float32)

    in_maps = []
    for k in range(cfg.n_cores):
        mods = slice(k * NM, (k + 1) * NM)
        w1p = np.zeros((NM, IN, HID), np.float32)
        w1p[:, :C_TRUE] = W1[mods, :C_TRUE]
        w1p[:, C_PAD:C_PAD + C_TRUE] = W1[mods, C_TRUE:CIN_TRUE]
        w1t = np.ascontiguousarray(w1p.transpose(0, 2, 1))
        wh = np.stack([w[mods] for w in Whs])            # [3, NM, HID, HID]
        wht = np.ascontiguousarray(wh.transpose(0, 1, 3, 2))
        w5c = np.ascontiguousarray(
            W5[mods, :, 0].reshape(NM, OC, 128).transpose(2, 0, 1)
        ).reshape(128, NM * OC)
        bias = np.ascontiguousarray(
            np.stack([b[mods, 0, :] for b in bs])        # [4l, NM, HID]
            .reshape(4, NM, OC, 128).transpose(3, 1, 0, 2))
        b5v = b5[mods, 0, 0].reshape(1, NM).astype(np.float32)
        in_maps.append({
            "geom": geom, "boxq": boxq, "maskdiv": maskdiv,
            "w1": w1p.astype(npdt), "w1t": w1t.astype(npdt),
            "wh": wh.astype(npdt), "wht": wht.astype(npdt),
            "w5c": w5c.astype(npdt), "w5f": w5c.astype(np.float32),
            "bias": bias.astype(np.float32), "b5v": b5v,
        })
    return in_maps, (pad, colvar)


def _assemble(results, inputs, cfg: Cfg):
    T = cfg.t_cols
    colvar = np.asarray(inputs["colvar_idx"], np.int32)
    fc = np.asarray(results[0]["fcontrib"], np.float32)   # [128,4,3,T]
    contrib = fc.transpose(3, 0, 1, 2).reshape(cfg.c_pad, 4, 3)[:cfg.c_true]
    forces = np.zeros((np.asarray(inputs["positions"]).shape[0], 3), np.float32)
    np.add.at(forces, colvar.reshape(-1), contrib.reshape(-1, 3))
    energy = np.float32(np.asarray(results[0]["energy"])[0, 0])
    return energy, forces


_NC_CACHE = {}


def _get_nc(cfg: Cfg):
    if cfg not in _NC_CACHE:
        _NC_CACHE[cfg] = _build_nc(cfg)
    return _NC_CACHE[cfg]


def run_raw(inputs, cfg: Cfg = REAL_CFG, trace: bool = False, tmpdir=None):
    """Run on hardware; returns ((energy, forces), BassKernelResults)."""
    nc = _get_nc(cfg)
    in_maps, _ = _make_in_maps(inputs, cfg)
    res = run_bass_kernel_spmd(
        nc, in_maps, core_ids=list(range(cfg.n_cores)), trace=trace,
        tmpdir=tmpdir)
    return _assemble(res.results, inputs, cfg), res


def kernel(**inputs):
    (energy, forces), _ = run_raw(inputs, REAL_CFG, trace=False)
    return energy, forces


# revision 35
# speedup vs baseline: 1.5481x; 1.5481x over previous
"""Trainium2 Bass kernel for nn_DihedralBiasVMap2 (dihedral-bias ensemble MLP).

Sharding: ensemble (model) axis of the MLP weights is split across 8 cores
(4 models each). Dihedral geometry + forward/backward of the local model
shard run on-device; per-model energy sums and jac statistics (sum, sum of
squares over models) are AllReduced on-device; every core then computes the
variance switch sigma and the sigma-scaled per-CV force contributions.
Host side does only data layout (gather of the 16k indexed atom rows,
padding/transpose of weights) and the final scatter-add of per-CV force
contributions into the full [1e6,3] forces array.

Compute layout notes:
- CV tiles are [128, T] with CV index c = t*128 + p (partition p, column t).
- The MLP runs on a single x vector per model; all matvecs use PE
  "orientation A": stationary lhsT = weight tile [K=128, M=128], moving
  rhs = activation column [128, 1], so activations stay in column layout
  [128, n_chunks] with zero transposes anywhere (backward uses pre-
  transposed weight copies uploaded from host).
- cos/sin of the torsion are computed algebraically (cos = x/r, sin = y/r
  from the dihedral dot/cross products) - no trig tables needed.
"""

import os
from contextlib import ExitStack
from dataclasses import dataclass

import numpy as np
import ml_dtypes

import concourse.bass as bass
import concourse.bacc as bacc
import concourse.mybir as mybir
import concourse.tile as tile
from concourse.bass_utils import run_bass_kernel_spmd

F32 = mybir.dt.float32
MUL = mybir.AluOpType.mult
ADD = mybir.AluOpType.add
SUB = mybir.AluOpType.subtract
ISGT = mybir.AluOpType.is_gt
ISGE = mybir.AluOpType.is_ge
ISLT = mybir.AluOpType.is_lt
MAXOP = mybir.AluOpType.max
MINOP = mybir.AluOpType.min

MAGIC = 12582912.0  # 1.5 * 2**23: float32 round-to-nearest-integer constant


@dataclass(frozen=True)
class Cfg:
    n_cores: int = 8
    nm: int = 4            # models per core
    t_cols: int = 32       # C_pad / 128
    oc: int = 4            # hidden / 128
    c_true: int = 4000
    e0: float = 2.0
    e1: float = 3.0
    cdt: str = "f32"       # "f32" or "bf16"

    @property
    def c_pad(self):
        return 128 * self.t_cols

    @property
    def ic1(self):
        return 2 * self.t_cols   # input chunks for layer 1 (IN = 2*c_pad)

    @property
    def in_dim(self):
        return 128 * self.ic1

    @property
    def hid(self):
        return 128 * self.oc

    @property
    def m_total(self):
        return self.n_cores * self.nm

    @property
    def mdt(self):
        return F32 if self.cdt == "f32" else mybir.dt.bfloat16

    @property
    def npdt(self):
        return np.float32 if self.cdt == "f32" else ml_dtypes.bfloat16


REAL_CFG = Cfg()


def _build_nc(cfg: Cfg):
    # Bacc (not raw Bass): its finalize runs the TRN2 lowering passes that
    # split multi-semaphore waits (HW allows one wait per instruction).
    nc = bacc.Bacc("TRN2")
    f32 = F32
    cdt = cfg.mdt
    T = cfg.t_cols
    OC = cfg.oc
    IC1 = cfg.ic1
    NM = cfg.nm
    HID = cfg.hid

    # ------------------- DRAM I/O -------------------
    geom_d = nc.dram_tensor("geom", [128, 4, 3, T], f32, kind="ExternalInput")
    boxq_d = nc.dram_tensor("boxq", [128, 2, 3, T], f32, kind="ExternalInput")
    maskdiv_d = nc.dram_tensor("maskdiv", [128, T], f32, kind="ExternalInput")
    w1_d = nc.dram_tensor("w1", [NM, cfg.in_dim, HID], cdt, kind="ExternalInput")
    w1t_d = nc.dram_tensor("w1t", [NM, HID, cfg.in_dim], cdt, kind="ExternalInput")
    wh_d = nc.dram_tensor("wh", [3, NM, HID, HID], cdt, kind="ExternalInput")
    wht_d = nc.dram_tensor("wht", [3, NM, HID, HID], cdt, kind="ExternalInput")
    w5c_d = nc.dram_tensor("w5c", [128, OC, NM], cdt, kind="ExternalInput")
    w5f_d = nc.dram_tensor("w5f", [128, OC, NM], f32, kind="ExternalInput")
    bias_d = nc.dram_tensor("bias", [128, 4, OC, NM], f32, kind="ExternalInput")
    b5v_d = nc.dram_tensor("b5v", [1, NM], f32, kind="ExternalInput")

    energy_d = nc.dram_tensor("energy", [1, 1], f32, kind="ExternalOutput")
    fcon_d = nc.dram_tensor("fcontrib", [128, 4, 3, T], f32, kind="ExternalOutput")

    with ExitStack() as ctx:
        tc = ctx.enter_context(tile.TileContext(nc))
        geo = ctx.enter_context(tc.tile_pool(name="geo", bufs=1))
        gtmp = ctx.enter_context(tc.tile_pool(name="gtmp", bufs=2))
        w1pool = ctx.enter_context(tc.tile_pool(name="w1pool", bufs=2))
        w1tpool = ctx.enter_context(tc.tile_pool(name="w1tpool", bufs=2))
        whpool = ctx.enter_context(tc.tile_pool(name="whpool", bufs=2))
        hpool = ctx.enter_context(tc.tile_pool(name="hpool", bufs=2))
        prowpool = ctx.enter_context(
            tc.tile_pool(name="prowpool", bufs=1, space="PSUM"))
        pcpool = ctx.enter_context(tc.tile_pool(name="pcpool", bufs=2, space="PSUM"))
        epool = ctx.enter_context(tc.tile_pool(name="epool", bufs=1, space="PSUM"))
        dram = ctx.enter_context(tc.tile_pool(name="dram", bufs=1, space="DRAM"))

        def vt(out, a, b, op):
            nc.vector.tensor_tensor(out=out, in0=a, in1=b, op=op)

        def vs(out, a, s1, s2=None, op0=MUL, op1=ADD):
            if s2 is None:
                nc.vector.tensor_scalar(out, a, s1, None, op0)
            else:
                nc.vector.tensor_scalar(out, a, s1, s2, op0, op1)

        def gtile(shape, name, dt=f32):
            return geo.tile(shape, dt, name=name, tag=name)

        def dot3(a, b, name):
            out = gtile([128, T], name)
            tmp = gtile([128, T], name + "_dt")
            vt(out, a[:, 0, :], b[:, 0, :], MUL)
            vt(tmp, a[:, 1, :], b[:, 1, :], MUL)
            vt(out, out, tmp, ADD)
            vt(tmp, a[:, 2, :], b[:, 2, :], MUL)
            vt(out, out, tmp, ADD)
            return out

        def cross3(a, b, name):
            out = gtile([128, 3, T], name)
            tmp = gtile([128, T], name + "_ct")
            for i, (i1, i2) in enumerate([(1, 2), (2, 0), (0, 1)]):
                vt(out[:, i, :], a[:, i1, :], b[:, i2, :], MUL)
                vt(tmp, a[:, i2, :], b[:, i1, :], MUL)
                vt(out[:, i, :], out[:, i, :], tmp, SUB)
            return out

        def refined_sqrt(s, name):
            """sqrt with one Newton step: y' = 0.5*(y + s/y)."""
            y = gtile(list(s.shape), name)
            nc.scalar.activation(y, s, mybir.ActivationFunctionType.Sqrt)
            iy = gtile(list(s.shape), name + "_i")
            nc.vector.reciprocal(iy, y)
            t = gtile(list(s.shape), name + "_n")
            vt(t, s, iy, MUL)
            vt(t, t, y, ADD)
            vs(t, t, 0.5)
            return t

        # ------------------- geometry -------------------
        g_sb = geo.tile([128, 4, 3, T], f32, name="g_sb", tag="g_sb")
        nc.sync.dma_start(out=g_sb, in_=geom_d[:, :, :, :])
        boxq_sb = geo.tile([128, 2, 3, T], f32, name="boxq_sb", tag="boxq_sb")
        nc.sync.dma_start(out=boxq_sb, in_=boxq_d[:, :, :, :])
        maskdiv_sb = geo.tile([128, T], f32, name="maskdiv_sb", tag="maskdiv_sb")
        nc.sync.dma_start(out=maskdiv_sb, in_=maskdiv_d[:, :])

        wp = []
        for a in range(4):
            q = gtile([128, 3, T], f"q{a}")
            vt(q, g_sb[:, a], boxq_sb[:, 1], MUL)           # p * invbox
            r = gtile([128, 3, T], f"r{a}")
            vs(r, q, MAGIC, -MAGIC, ADD, ADD)               # round-to-int
            gt = gtile([128, 3, T], f"gt{a}")
            vt(gt, r, q, ISGT)
            vt(r, r, gt, SUB)                               # floor(q)
            vt(r, r, boxq_sb[:, 0], MUL)                    # floor*box
            w = gtile([128, 3, T], f"wp{a}")
            vt(w, g_sb[:, a], r, SUB)                       # wrapped coords
            wp.append(w)

        b1 = gtile([128, 3, T], "b1")
        b2 = gtile([128, 3, T], "b2")
        b3 = gtile([128, 3, T], "b3")
        vt(b1, wp[1], wp[0], SUB)
        vt(b2, wp[2], wp[1], SUB)
        vt(b3, wp[3], wp[2], SUB)

        n1 = cross3(b1, b2, "n1")
        n2 = cross3(b2, b3, "n2")
        tnn = cross3(n2, n1, "tnn")

        xd = dot3(n1, n2, "xd")          # = |n1||n2| cos(phi)
        yd0 = dot3(tnn, b2, "yd0")       # = |n1||n2||b2| sin(phi)
        s2d = dot3(b2, b2, "s2d")
        n1sq = dot3(n1, n1, "n1sq")
        n2sq = dot3(n2, n2, "n2sq")
        d12 = dot3(b1, b2, "d12")
        d32 = dot3(b3, b2, "d32")

        nb2 = refined_sqrt(s2d, "nb2")
        inb2 = gtile([128, T], "inb2")
        nc.vector.reciprocal(inb2, nb2)
        ib2 = gtile([128, T], "ib2")
        nc.vector.reciprocal(ib2, s2d)
        yd = gtile([128, T], "yd")
        vt(yd, yd0, inb2, MUL)

        r2 = gtile([128, T], "r2")
        tmpg = gtile([128, T], "tmpg")
        vt(r2, xd, xd, MUL)
        vt(tmpg, yd, yd, MUL)
        vt(r2, r2, tmpg, ADD)
        rr = refined_sqrt(r2, "rr")
        ir = gtile([128, T], "ir")
        nc.vector.reciprocal(ir, rr)
        cos_t = gtile([128, T], "cos_t")
        sin_t = gtile([128, T], "sin_t")
        vt(cos_t, xd, ir, MUL)
        vt(sin_t, yd, ir, MUL)

        i1 = gtile([128, T], "i1")
        i2 = gtile([128, T], "i2")
        nc.vector.reciprocal(i1, n1sq)
        nc.vector.reciprocal(i2, n2sq)
        A = gtile([128, T], "Acf")
        B = gtile([128, T], "Bcf")
        vt(A, d12, ib2, MUL)
        vt(B, d32, ib2, MUL)
        cF = gtile([128, T], "cF")
        cH = gtile([128, T], "cH")
        vt(cF, nb2, i1, MUL)
        vt(cH, nb2, i2, MUL)
        Fc = gtile([128, 3, T], "Fc")
        Hc = gtile([128, 3, T], "Hc")
        for c in range(3):
            vt(Fc[:, c, :], n1[:, c, :], cF, MUL)
            vt(Hc[:, c, :], n2[:, c, :], cH, MUL)
        cA = gtile([128, T], "cA")      # -(1+A)
        vs(cA, A, -1.0, -1.0, MUL, ADD)
        cB = gtile([128, T], "cB")      # 1+B
        vs(cB, B, 1.0, None, ADD)
        g1 = gtile([128, 3, T], "g1")
        g2 = gtile([128, 3, T], "g2")
        t1g = gtile([128, T], "t1g")
        for c in range(3):
            vt(g1[:, c, :], Fc[:, c, :], cA, MUL)
            vt(t1g, Hc[:, c, :], B, MUL)
            vt(g1[:, c, :], g1[:, c, :], t1g, SUB)
            vt(g2[:, c, :], Fc[:, c, :], A, MUL)
            vt(t1g, Hc[:, c, :], cB, MUL)
            vt(g2[:, c, :], g2[:, c, :], t1g, ADD)

        # x vector in compute dtype, column layout [128, 2T]
        x_sb = geo.tile([128, IC1], cdt, name="x_sb", tag="x_sb")
        nc.vector.tensor_copy(out=x_sb[:, 0:T], in_=cos_t)
        nc.vector.tensor_copy(out=x_sb[:, T:2 * T], in_=sin_t)

        # ------------------- small params -------------------
        w5c_sb = geo.tile([128, OC, NM], cdt, name="w5c_sb", tag="w5c_sb")
        nc.sync.dma_start(out=w5c_sb, in_=w5c_d[:, :, :])
        w5f_sb = geo.tile([128, OC, NM], f32, name="w5f_sb", tag="w5f_sb")
        nc.sync.dma_start(out=w5f_sb, in_=w5f_d[:, :, :])
        bias_sb = geo.tile([128, 4, OC, NM], f32, name="bias_sb", tag="bias_sb")
        nc.sync.dma_start(out=bias_sb, in_=bias_d[:, :, :, :])
        b5v_sb = geo.tile([1, NM], f32, name="b5v_sb", tag="b5v_sb")
        nc.sync.dma_start(out=b5v_sb, in_=b5v_d[:, :])

        s1 = geo.tile([128, T], f32, name="s1", tag="s1")
        s2t = geo.tile([128, T], f32, name="s2t", tag="s2t")
        nc.vector.memset(s1, 0.0)
        nc.vector.memset(s2t, 0.0)

        idN = geo.tile([128, 128], f32, name="idN", tag="idN")
        from concourse.masks import make_identity
        make_identity(nc, idN)

        pe_ps = epool.tile([1, NM], f32, name="pe_ps", tag="pe_ps")

        # ------------------- MLP fwd+bwd (weights as moving operand) -----
        # All matvecs stream the WEIGHT tile as the moving operand (N=HID
        # wide) with the activation chunk stationary. Results land as per-
        # model rows in a shared [NM, 512] PSUM tile (sequential accumulation
        # groups), then one ACT copy + PE transposes ([NM,128] -> [128,NM]
        # against a small identity) return them to column layout.
        GJ = min(8, IC1)        # layer-1 i-chunks per DMA
        BW = min(512, 128 * IC1)  # backward sweep i-block width
        CPB = BW // 128
        NBLK = (128 * IC1) // BW

        # Each model's result row is a [1, 512] PSUM tile in its own bank
        # (PE outputs start at partition 0). The rows are then stacked into
        # one SBUF tile at partitions 32*m (legal compute-engine bases) and
        # PE-transposed [128,128]; valid model columns come out at free-dim
        # columns 32*m (strided AP for consumers). The stack tile is zeroed
        # per round so the transpose only sees finite values.
        def alloc_rowblk():
            return [prowpool.tile([1, 512], f32, name=f"rowm{m}", tag=f"rowm{m}")
                    for m in range(NM)]

        def rows_to_cols(rows, width, consume):
            rowsb = gtmp.tile([128, 512], f32, name="rowsb", tag="rowsb")
            nc.vector.memset(rowsb, 0.0)
            for m in range(NM):
                nc.scalar.copy(rowsb[32 * m:32 * m + 1, 0:width],
                               rows[m][0:1, 0:width])
            for j in range(width // 128):
                ptr = pcpool.tile([128, 128], f32, name="ptr", tag="ptr")
                nc.tensor.transpose(ptr, rowsb[:, j * 128:(j + 1) * 128], idN)
                pcols = ptr.rearrange("p (n q) -> p n q", q=32)[:, :, 0]
                consume(j, pcols[:, 0:NM])

        h_alls = []

        # --- layer 1 forward: z1 rows, model-sequential groups
        z1blk = alloc_rowblk()
        for m in range(NM):
            w1r = w1_d[m].rearrange("(c p) o -> p c o", p=128)
            for j in range(IC1 // GJ):
                w1sb = w1pool.tile([128, GJ, HID], cdt, name="w1sb", tag="w1sb")
                nc.sync.dma_start(out=w1sb, in_=w1r[:, j * GJ:(j + 1) * GJ, :])
                for c in range(GJ):
                    jj = j * GJ + c
                    nc.tensor.matmul(
                        z1blk[m][0:1, 0:HID],
                        x_sb[:, jj:jj + 1],
                        w1sb[:, c, :],
                        start=(jj == 0), stop=(jj == IC1 - 1),
                    )

        def make_h(l, prow):
            h_all = geo.tile([128, OC, NM], cdt, name=f"h{l}", tag=f"h{l}")
            zf = gtmp.tile([128, NM], f32, name="zf", tag="zf")

            def consume(j, pcols):
                vt(zf, pcols, bias_sb[:, l, j, :], ADD)
                nc.vector.tensor_relu(h_all[:, j, :], zf)

            rows_to_cols(prow, HID, consume)
            h_alls.append(h_all)

        make_h(0, z1blk)

        # --- hidden layers forward (model-lockstep per layer)
        for l in range(3):
            whsbs = []
            for m in range(NM):
                whsb = whpool.tile([128, OC, HID], cdt,
                                   name=f"whm{m}", tag=f"whm{m}")
                nc.sync.dma_start(
                    out=whsb, in_=wh_d[l, m].rearrange("(c p) o -> p c o", p=128))
                whsbs.append(whsb)
            zblk = alloc_rowblk()
            for m in range(NM):
                for ic in range(OC):
                    nc.tensor.matmul(
                        zblk[m][0:1, 0:HID],
                        h_alls[l][:, ic, m:m + 1],
                        whsbs[m][:, ic, :],
                        start=(ic == 0), stop=(ic == OC - 1),
                    )
            make_h(l + 1, zblk)

        # --- energy readout + backward seed
        h4 = h_alls[3]
        for m in range(NM):
            for j in range(OC):
                nc.tensor.matmul(
                    pe_ps[0:1, m:m + 1],
                    w5c_sb[:, j, m:m + 1],
                    h4[:, j, m:m + 1],
                    start=(j == 0), stop=(j == OC - 1),
                )
        mask = gtmp.tile([128, OC, NM], f32, name="mask", tag="mask")
        nc.vector.tensor_scalar(mask, h4, 0.0, None, ISGT)
        gz_all = hpool.tile([128, OC, NM], cdt, name="gz_all", tag="gz_all")
        vt(gz_all, w5f_sb, mask, MUL)

        # --- hidden layers backward (lockstep)
        for l in [2, 1, 0]:
            whtsbs = []
            for m in range(NM):
                whtsb = whpool.tile([128, OC, HID], cdt,
                                    name=f"whtm{m}", tag=f"whm{m}")
                nc.sync.dma_start(
                    out=whtsb, in_=wht_d[l, m].rearrange("(c p) i -> p c i", p=128))
                whtsbs.append(whtsb)
            gblk = alloc_rowblk()
            for m in range(NM):
                for o in range(OC):
                    nc.tensor.matmul(
                        gblk[m][0:1, 0:HID],
                        gz_all[:, o, m:m + 1],
                        whtsbs[m][:, o, :],
                        start=(o == 0), stop=(o == OC - 1),
                    )
            mask = gtmp.tile([128, OC, NM], f32, name="mask", tag="mask")
            nc.vector.tensor_scalar(mask, h_alls[l], 0.0, None, ISGT)
            gz_next = hpool.tile([128, OC, NM], cdt, name="gz_all", tag="gz_all")

            def consume_g(j, pcols, gz_next=gz_next, mask=mask):
                vt(gz_next[:, j, :], pcols, mask[:, j, :], MUL)

            rows_to_cols(gblk, HID, consume_g)
            gz_all = gz_next

        # --- layer-1 backward sweep -> gx columns for all models
        gxall = geo.tile([128, IC1, NM], f32, name="gxall", tag="gxall")
        w1trs = [w1t_d[m].rearrange("(c p) i -> p c i", p=128)
                 for m in range(NM)]
        for ib in range(NBLK):
            w1tblk = w1tpool.tile([128, NM, OC, BW], cdt,
                                  name="w1tblk", tag="w1tblk")
            for m in range(NM):
                for o in range(OC):
                    nc.sync.dma_start(
                        out=w1tblk[:, m, o, :],
                        in_=w1trs[m][:, o, ib * BW:(ib + 1) * BW])
            gxblk = alloc_rowblk()
            for m in range(NM):
                for o in range(OC):
                    nc.tensor.matmul(
                        gxblk[m][0:1, 0:BW],
                        gz_all[:, o, m:m + 1],
                        w1tblk[:, m, o, :],
                        start=(o == 0), stop=(o == OC - 1),
                    )

            def consume_gx(j, pcols, ib=ib):
                nc.vector.tensor_copy(gxall[:, ib * CPB + j, :], pcols)

            rows_to_cols(gxblk, BW, consume_gx)

        # --- jac + variance statistics per model
        for m in range(NM):
            jac = gtmp.tile([128, T], f32, name="jac", tag="jac")
            jtmp = gtmp.tile([128, T], f32, name="jtmp", tag="jtmp")
            vt(jac, cos_t, gxall[:, T:2 * T, m], MUL)
            vt(jtmp, sin_t, gxall[:, 0:T, m], MUL)
            vt(jac, jac, jtmp, SUB)
            vt(s1, s1, jac, ADD)
            vt(jtmp, jac, jac, MUL)
            vt(s2t, s2t, jtmp, ADD)

        # ------------------- cross-core reduction -------------------
        el = geo.tile([1, NM], f32, name="el", tag="el")
        vt(el, pe_ps[0:1, :], b5v_sb, ADD)
        ecol = geo.tile([128, 1], f32, name="ecol", tag="ecol")
        nc.vector.memset(ecol, 0.0)
        nc.vector.tensor_reduce(ecol[0:1, 0:1], el, mybir.AxisListType.X, ADD)

        ccin = dram.tile([128, 2 * T + 1], f32, name="ccin")
        cc_addr = "Shared" if cfg.n_cores > 4 else "Local"
        ccout = dram.tile([128, 2 * T + 1], f32, name="ccout", addr_space=cc_addr)
        nc.sync.dma_start(out=ccin[:, 0:T], in_=s1)
        nc.sync.dma_start(out=ccin[:, T:2 * T], in_=s2t)
        nc.sync.dma_start(out=ccin[:, 2 * T:2 * T + 1], in_=ecol)
        nc.gpsimd.collective_compute(
            "AllReduce",
            ADD,
            replica_groups=[list(range(cfg.n_cores))],
            ins=[ccin.opt()],
            outs=[ccout.opt()],
        )
        cc_sb = geo.tile([128, 2 * T + 1], f32, name="cc_sb", tag="cc_sb")
        nc.sync.dma_start(out=cc_sb, in_=ccout[:, :])

        # ------------------- sigma switch + outputs -------------------
        M = float(cfg.m_total)
        gbar = geo.tile([128, T], f32, name="gbar", tag="gbar")
        vs(gbar, cc_sb[:, 0:T], 1.0 / M)
        vvar = geo.tile([128, T], f32, name="vvar", tag="vvar")
        vt(vvar, gbar, gbar, MUL)
        nc.vector.scalar_tensor_tensor(
            vvar, vvar, -M, cc_sb[:, T:2 * T], MUL, ADD)   # S2 - M*gbar^2
        vt(vvar, vvar, maskdiv_sb, MUL)                    # mask / (M-1)
        rowsum = geo.tile([128, 1], f32, name="rowsum", tag="rowsum")
        nc.vector.tensor_reduce(rowsum, vvar, mybir.AxisListType.X, ADD)
        ones_t = geo.tile([128, 1], f32, name="ones_t", tag="ones_t")
        nc.vector.memset(ones_t, 1.0)
        tot_ps = epool.tile([1, 1], f32, name="tot_ps", tag="tot_ps")
        nc.tensor.matmul(tot_ps[0:1, 0:1], ones_t, rowsum, start=True, stop=True)
        md = geo.tile([1, 1], f32, name="md", tag="md")
        nc.scalar.activation(md, tot_ps, mybir.ActivationFunctionType.Sqrt,
                             scale=1.0 / cfg.c_true)
        # iswitch and smooth switch
        den = cfg.e1 - cfg.e0
        isw = geo.tile([1, 1], f32, name="isw", tag="isw")
        vs(isw, md, -1.0 / den, cfg.e1 / den, MUL, ADD)
        arg = geo.tile([1, 1], f32, name="arg", tag="arg")
        pi = float(np.pi)
        vs(arg, md, pi / den, pi / 2 - pi * cfg.e1 / den, MUL, ADD)
        vs(arg, arg, -pi / 2, pi / 2, MAXOP, MINOP)
        sinv = geo.tile([1, 1], f32, name="sinv", tag="sinv")
        nc.scalar.activation(sinv, arg, mybir.ActivationFunctionType.Sin)
        smooth = geo.tile([1, 1], f32, name="smooth", tag="smooth")
        vs(smooth, sinv, -0.5, 0.5, MUL, ADD)
        ge1 = geo.tile([1, 1], f32, name="ge1", tag="ge1")
        nc.vector.tensor_scalar(ge1, isw, 1.0, None, ISGE)
        lt0 = geo.tile([1, 1], f32, name="lt0", tag="lt0")
        nc.vector.tensor_scalar(lt0, isw, 0.0, None, ISLT)
        na = geo.tile([1, 1], f32, name="na", tag="na")
        vs(na, ge1, -1.0, 1.0, MUL, ADD)
        nb = geo.tile([1, 1], f32, name="nb", tag="nb")
        vs(nb, lt0, -1.0, 1.0, MUL, ADD)
        sig = geo.tile([1, 1], f32, name="sig", tag="sig")
        vt(sig, na, nb, MUL)
        vt(sig, sig, smooth, MUL)
        vt(sig, sig, ge1, ADD)

        eout = geo.tile([1, 1], f32, name="eout", tag="eout")
        vs(eout, cc_sb[0:1, 2 * T:2 * T + 1], 1.0 / M)
        vt(eout, eout, sig, MUL)
        nc.sync.dma_start(out=energy_d[:, :], in_=eout)

        # broadcast sigma across partitions via DRAM bounce
        sigd = dram.tile([1, 1], f32, name="sigd")
        nc.sync.dma_start(out=sigd[:, :], in_=sig)
        sigb = geo.tile([128, 1], f32, name="sigb", tag="sigb")
        nc.sync.dma_start(out=sigb, in_=sigd.to_broadcast((128, 1)))

        wv = geo.tile([128, T], f32, name="wv", tag="wv")
        nc.vector.tensor_scalar(wv, gbar, sigb, None, MUL)
        wneg = geo.tile([128, T], f32, name="wneg", tag="wneg")
        vs(wneg, wv, -1.0)
        fcon = geo.tile([128, 4, 3, T], f32, name="fcon", tag="fcon")
        for c in range(3):
            vt(fcon[:, 0, c, :], Fc[:, c, :], wv, MUL)
            vt(fcon[:, 1, c, :], g1[:, c, :], wv, MUL)
            vt(fcon[:, 2, c, :], g2[:, c, :], wv, MUL)
            vt(fcon[:, 3, c, :], Hc[:, c, :], wneg, MUL)
        nc.sync.dma_start(out=fcon_d[:, :, :, :], in_=fcon)

    nc.finalize()   # Bacc: runs lowering passes (reg alloc, wait splitting)
    return nc


# ---------------------------------------------------------------------------
# host-side data prep
# ---------------------------------------------------------------------------

def _make_in_maps(inputs, cfg: Cfg):
    npdt = cfg.npdt
    T = cfg.t_cols
    NM = cfg.nm
    OC = cfg.oc
    HID = cfg.hid
    C_PAD = cfg.c_pad
    C_TRUE = cfg.c_true
    IN = cfg.in_dim
    CIN_TRUE = 2 * C_TRUE

    positions = np.asarray(inputs["positions"], np.float32)
    colvar = np.asarray(inputs["colvar_idx"], np.int32)
    box = np.diag(np.asarray(inputs["boxvectors"], np.float32)).copy()

    # padded CV index table (repeat row 0; padded rows are masked out)
    pad = np.zeros((C_PAD, 4), np.int32)
    pad[:C_TRUE] = colvar
    pad[C_TRUE:] = colvar[0]
    flat = pad.reshape(-1)
    sel = positions[flat].reshape(C_PAD, 4, 3)           # host gather
    geom = np.ascontiguousarray(
        sel.reshape(T, 128, 4, 3).transpose(1, 2, 3, 0)).astype(np.float32)

    boxq = np.empty((128, 2, 3, T), np.float32)
    boxq[:, 0] = box[None, :, None]
    boxq[:, 1] = (np.float32(1.0) / box)[None, :, None]

    cv_idx = (np.arange(T)[None, :] * 128 + np.arange(128)[:, None])
    maskdiv = ((cv_idx < C_TRUE).astype(np.float32)
               / np.float32(cfg.m_total - 1))

    W1 = np.asarray(inputs["W1"], np.float32)
    Whs = [np.asarray(inputs[f"W{i}"], np.float32) for i in (2, 3, 4)]
    W5 = np.asarray(inputs["W5"], np.float32)
    bs = [np.asarray(inputs[f"b{i}"], np.float32) for i in (1, 2, 3, 4)]
    b5 = np.asarray(inputs["b5"], np.float32)

    in_maps = []
    for k in range(cfg.n_cores):
        mods = slice(k * NM, (k + 1) * NM)
        w1p = np.zeros((NM, IN, HID), np.float32)
        w1p[:, :C_TRUE] = W1[mods, :C_TRUE]
        w1p[:, C_PAD:C_PAD + C_TRUE] = W1[mods, C_TRUE:CIN_TRUE]
        w1t = np.ascontiguousarray(w1p.transpose(0, 2, 1))
        wh = np.stack([w[mods] for w in Whs])            # [3, NM, HID, HID]
        wht = np.ascontiguousarray(wh.transpose(0, 1, 3, 2))
        w5c = np.ascontiguousarray(
            W5[mods, :, 0].reshape(NM, OC, 128).transpose(2, 1, 0))  # [128,OC,NM]
        bias = np.ascontiguousarray(
            np.stack([b[mods, 0, :] for b in bs])        # [4l, NM, HID]
            .reshape(4, NM, OC, 128).transpose(3, 0, 2, 1))  # [128,4,OC,NM]
        b5v = b5[mods, 0, 0].reshape(1, NM).astype(np.float32)
        in_maps.append({
            "geom": geom, "boxq": boxq, "maskdiv": maskdiv,
            "w1": w1p.astype(npdt), "w1t": w1t.astype(npdt),
            "wh": wh.astype(npdt), "wht": wht.astype(npdt),
            "w5c": w5c.astype(npdt), "w5f": w5c.astype(np.float32),
            "bias": bias.astype(np.float32), "b5v": b5v,
        })
    return in_maps, (pad, colvar)


def _assemble(results, inputs, cfg: Cfg):
    T = cfg.t_cols
    colvar = np.asarray(inputs["colvar_idx"], np.int32)
    fc = np.asarray(results[0]["fcontrib"], np.float32)   # [128,4,3,T]
    contrib = fc.transpose(3, 0, 1, 2).reshape(cfg.c_pad, 4, 3)[:cfg.c_true]
    forces = np.zeros((np.asarray(inputs["positions"]).shape[0], 3), np.float32)
    np.add.at(forces, colvar.reshape(-1), contrib.reshape(-1, 3))
    energy = np.float32(np.asarray(results[0]["energy"])[0, 0])
    return energy, forces


_NC_CACHE = {}


def _get_nc(cfg: Cfg):
    if cfg not in _NC_CACHE:
        _NC_CACHE[cfg] = _build_nc(cfg)
    return _NC_CACHE[cfg]


def run_raw(inputs, cfg: Cfg = REAL_CFG, trace: bool = False, tmpdir=None):
    """Run on hardware; returns ((energy, forces), BassKernelResults)."""
    nc = _get_nc(cfg)
    in_maps, _ = _make_in_maps(inputs, cfg)
    res = run_bass_kernel_spmd(
        nc, in_maps, core_ids=list(range(cfg.n_cores)), trace=trace,
        tmpdir=tmpdir)
    return _assemble(res.results, inputs, cfg), res


def kernel(**inputs):
    (energy, forces), _ = run_raw(inputs, REAL_CFG, trace=False)
    return energy, forces


# revision 36
# speedup vs baseline: 2.5957x; 1.6767x over previous
"""Trainium2 Bass kernel for nn_DihedralBiasVMap2 (dihedral-bias ensemble MLP).

Sharding: ensemble (model) axis of the MLP weights is split across 8 cores
(4 models each). Dihedral geometry + forward/backward of the local model
shard run on-device; per-model energy sums and jac statistics (sum, sum of
squares over models) are AllReduced on-device; every core then computes the
variance switch sigma and the sigma-scaled per-CV force contributions.
Host side does only data layout (gather of the 16k indexed atom rows,
padding/transpose of weights) and the final scatter-add of per-CV force
contributions into the full [1e6,3] forces array.

Compute layout notes:
- CV tiles are [128, T] with CV index c = t*128 + p (partition p, column t).
- The MLP runs on a single x vector per model; all matvecs use PE
  "orientation A": stationary lhsT = weight tile [K=128, M=128], moving
  rhs = activation column [128, 1], so activations stay in column layout
  [128, n_chunks] with zero transposes anywhere (backward uses pre-
  transposed weight copies uploaded from host).
- cos/sin of the torsion are computed algebraically (cos = x/r, sin = y/r
  from the dihedral dot/cross products) - no trig tables needed.
"""

import os
from contextlib import ExitStack
from dataclasses import dataclass

import numpy as np
import ml_dtypes

import concourse.bass as bass
import concourse.bacc as bacc
import concourse.mybir as mybir
import concourse.tile as tile
from concourse.bass_utils import run_bass_kernel_spmd

F32 = mybir.dt.float32
MUL = mybir.AluOpType.mult
ADD = mybir.AluOpType.add
SUB = mybir.AluOpType.subtract
ISGT = mybir.AluOpType.is_gt
ISGE = mybir.AluOpType.is_ge
ISLT = mybir.AluOpType.is_lt
MAXOP = mybir.AluOpType.max
MINOP = mybir.AluOpType.min

MAGIC = 12582912.0  # 1.5 * 2**23: float32 round-to-nearest-integer constant


@dataclass(frozen=True)
class Cfg:
    n_cores: int = 8
    nm: int = 4            # models per core
    t_cols: int = 32       # C_pad / 128
    oc: int = 4            # hidden / 128
    c_true: int = 4000
    e0: float = 2.0
    e1: float = 3.0
    cdt: str = "f32"       # "f32" or "bf16"

    @property
    def c_pad(self):
        return 128 * self.t_cols

    @property
    def ic1(self):
        return 2 * self.t_cols   # input chunks for layer 1 (IN = 2*c_pad)

    @property
    def in_dim(self):
        return 128 * self.ic1

    @property
    def hid(self):
        return 128 * self.oc

    @property
    def m_total(self):
        return self.n_cores * self.nm

    @property
    def mdt(self):
        return F32 if self.cdt == "f32" else mybir.dt.bfloat16

    @property
    def npdt(self):
        return np.float32 if self.cdt == "f32" else ml_dtypes.bfloat16


REAL_CFG = Cfg()


def _build_nc(cfg: Cfg):
    # Bacc (not raw Bass): its finalize runs the TRN2 lowering passes that
    # split multi-semaphore waits (HW allows one wait per instruction).
    nc = bacc.Bacc("TRN2")
    f32 = F32
    cdt = cfg.mdt
    T = cfg.t_cols
    OC = cfg.oc
    IC1 = cfg.ic1
    NM = cfg.nm
    HID = cfg.hid

    # ------------------- DRAM I/O -------------------
    geom_d = nc.dram_tensor("geom", [128, 4, 3, T], f32, kind="ExternalInput")
    boxq_d = nc.dram_tensor("boxq", [128, 2, 3, T], f32, kind="ExternalInput")
    maskdiv_d = nc.dram_tensor("maskdiv", [128, T], f32, kind="ExternalInput")
    w1_d = nc.dram_tensor("w1", [NM, cfg.in_dim, HID], cdt, kind="ExternalInput")
    w1t_d = nc.dram_tensor("w1t", [NM, HID, cfg.in_dim], cdt, kind="ExternalInput")
    wh_d = nc.dram_tensor("wh", [3, NM, HID, HID], cdt, kind="ExternalInput")
    wht_d = nc.dram_tensor("wht", [3, NM, HID, HID], cdt, kind="ExternalInput")
    w5c_d = nc.dram_tensor("w5c", [128, OC, NM], cdt, kind="ExternalInput")
    w5f_d = nc.dram_tensor("w5f", [128, OC, NM], f32, kind="ExternalInput")
    bias_d = nc.dram_tensor("bias", [128, 4, OC, NM], f32, kind="ExternalInput")
    b5v_d = nc.dram_tensor("b5v", [1, NM], f32, kind="ExternalInput")

    energy_d = nc.dram_tensor("energy", [1, 1], f32, kind="ExternalOutput")
    fcon_d = nc.dram_tensor("fcontrib", [128, 4, 3, T], f32, kind="ExternalOutput")

    with ExitStack() as ctx:
        tc = ctx.enter_context(tile.TileContext(nc))
        geo = ctx.enter_context(tc.tile_pool(name="geo", bufs=1))
        gtmp = ctx.enter_context(tc.tile_pool(name="gtmp", bufs=2))
        w1pool = ctx.enter_context(tc.tile_pool(name="w1pool", bufs=2))
        w1tpool = ctx.enter_context(tc.tile_pool(name="w1tpool", bufs=2))
        whpool = ctx.enter_context(tc.tile_pool(name="whpool", bufs=2))
        hpool = ctx.enter_context(tc.tile_pool(name="hpool", bufs=2))
        prowpool = ctx.enter_context(
            tc.tile_pool(name="prowpool", bufs=1, space="PSUM"))
        pcpool = ctx.enter_context(tc.tile_pool(name="pcpool", bufs=2, space="PSUM"))
        epool = ctx.enter_context(tc.tile_pool(name="epool", bufs=1, space="PSUM"))
        dram = ctx.enter_context(tc.tile_pool(name="dram", bufs=1, space="DRAM"))

        def vt(out, a, b, op):
            nc.vector.tensor_tensor(out=out, in0=a, in1=b, op=op)

        def vs(out, a, s1, s2=None, op0=MUL, op1=ADD):
            if s2 is None:
                nc.vector.tensor_scalar(out, a, s1, None, op0)
            else:
                nc.vector.tensor_scalar(out, a, s1, s2, op0, op1)

        def gtile(shape, name, dt=f32):
            return geo.tile(shape, dt, name=name, tag=name)

        def dot3(a, b, name):
            out = gtile([128, T], name)
            tmp = gtile([128, T], name + "_dt")
            vt(out, a[:, 0, :], b[:, 0, :], MUL)
            vt(tmp, a[:, 1, :], b[:, 1, :], MUL)
            vt(out, out, tmp, ADD)
            vt(tmp, a[:, 2, :], b[:, 2, :], MUL)
            vt(out, out, tmp, ADD)
            return out

        def cross3(a, b, name):
            out = gtile([128, 3, T], name)
            tmp = gtile([128, T], name + "_ct")
            for i, (i1, i2) in enumerate([(1, 2), (2, 0), (0, 1)]):
                vt(out[:, i, :], a[:, i1, :], b[:, i2, :], MUL)
                vt(tmp, a[:, i2, :], b[:, i1, :], MUL)
                vt(out[:, i, :], out[:, i, :], tmp, SUB)
            return out

        def refined_sqrt(s, name):
            """sqrt with one Newton step: y' = 0.5*(y + s/y)."""
            y = gtile(list(s.shape), name)
            nc.scalar.activation(y, s, mybir.ActivationFunctionType.Sqrt)
            iy = gtile(list(s.shape), name + "_i")
            nc.vector.reciprocal(iy, y)
            t = gtile(list(s.shape), name + "_n")
            vt(t, s, iy, MUL)
            vt(t, t, y, ADD)
            vs(t, t, 0.5)
            return t

        # ------------------- geometry -------------------
        g_sb = geo.tile([128, 4, 3, T], f32, name="g_sb", tag="g_sb")
        nc.sync.dma_start(out=g_sb, in_=geom_d[:, :, :, :])
        boxq_sb = geo.tile([128, 2, 3, T], f32, name="boxq_sb", tag="boxq_sb")
        nc.sync.dma_start(out=boxq_sb, in_=boxq_d[:, :, :, :])
        maskdiv_sb = geo.tile([128, T], f32, name="maskdiv_sb", tag="maskdiv_sb")
        nc.sync.dma_start(out=maskdiv_sb, in_=maskdiv_d[:, :])

        wp = []
        for a in range(4):
            q = gtile([128, 3, T], f"q{a}")
            vt(q, g_sb[:, a], boxq_sb[:, 1], MUL)           # p * invbox
            r = gtile([128, 3, T], f"r{a}")
            vs(r, q, MAGIC, -MAGIC, ADD, ADD)               # round-to-int
            gt = gtile([128, 3, T], f"gt{a}")
            vt(gt, r, q, ISGT)
            vt(r, r, gt, SUB)                               # floor(q)
            vt(r, r, boxq_sb[:, 0], MUL)                    # floor*box
            w = gtile([128, 3, T], f"wp{a}")
            vt(w, g_sb[:, a], r, SUB)                       # wrapped coords
            wp.append(w)

        b1 = gtile([128, 3, T], "b1")
        b2 = gtile([128, 3, T], "b2")
        b3 = gtile([128, 3, T], "b3")
        vt(b1, wp[1], wp[0], SUB)
        vt(b2, wp[2], wp[1], SUB)
        vt(b3, wp[3], wp[2], SUB)

        n1 = cross3(b1, b2, "n1")
        n2 = cross3(b2, b3, "n2")
        tnn = cross3(n2, n1, "tnn")

        xd = dot3(n1, n2, "xd")          # = |n1||n2| cos(phi)
        yd0 = dot3(tnn, b2, "yd0")       # = |n1||n2||b2| sin(phi)
        s2d = dot3(b2, b2, "s2d")
        n1sq = dot3(n1, n1, "n1sq")
        n2sq = dot3(n2, n2, "n2sq")
        d12 = dot3(b1, b2, "d12")
        d32 = dot3(b3, b2, "d32")

        nb2 = refined_sqrt(s2d, "nb2")
        inb2 = gtile([128, T], "inb2")
        nc.vector.reciprocal(inb2, nb2)
        ib2 = gtile([128, T], "ib2")
        nc.vector.reciprocal(ib2, s2d)
        yd = gtile([128, T], "yd")
        vt(yd, yd0, inb2, MUL)

        r2 = gtile([128, T], "r2")
        tmpg = gtile([128, T], "tmpg")
        vt(r2, xd, xd, MUL)
        vt(tmpg, yd, yd, MUL)
        vt(r2, r2, tmpg, ADD)
        rr = refined_sqrt(r2, "rr")
        ir = gtile([128, T], "ir")
        nc.vector.reciprocal(ir, rr)
        cos_t = gtile([128, T], "cos_t")
        sin_t = gtile([128, T], "sin_t")
        vt(cos_t, xd, ir, MUL)
        vt(sin_t, yd, ir, MUL)

        i1 = gtile([128, T], "i1")
        i2 = gtile([128, T], "i2")
        nc.vector.reciprocal(i1, n1sq)
        nc.vector.reciprocal(i2, n2sq)
        A = gtile([128, T], "Acf")
        B = gtile([128, T], "Bcf")
        vt(A, d12, ib2, MUL)
        vt(B, d32, ib2, MUL)
        cF = gtile([128, T], "cF")
        cH = gtile([128, T], "cH")
        vt(cF, nb2, i1, MUL)
        vt(cH, nb2, i2, MUL)
        Fc = gtile([128, 3, T], "Fc")
        Hc = gtile([128, 3, T], "Hc")
        for c in range(3):
            vt(Fc[:, c, :], n1[:, c, :], cF, MUL)
            vt(Hc[:, c, :], n2[:, c, :], cH, MUL)
        cA = gtile([128, T], "cA")      # -(1+A)
        vs(cA, A, -1.0, -1.0, MUL, ADD)
        cB = gtile([128, T], "cB")      # 1+B
        vs(cB, B, 1.0, None, ADD)
        g1 = gtile([128, 3, T], "g1")
        g2 = gtile([128, 3, T], "g2")
        t1g = gtile([128, T], "t1g")
        for c in range(3):
            vt(g1[:, c, :], Fc[:, c, :], cA, MUL)
            vt(t1g, Hc[:, c, :], B, MUL)
            vt(g1[:, c, :], g1[:, c, :], t1g, SUB)
            vt(g2[:, c, :], Fc[:, c, :], A, MUL)
            vt(t1g, Hc[:, c, :], cB, MUL)
            vt(g2[:, c, :], g2[:, c, :], t1g, ADD)

        # x vector in compute dtype, column layout [128, 2T]
        x_sb = geo.tile([128, IC1], cdt, name="x_sb", tag="x_sb")
        nc.vector.tensor_copy(out=x_sb[:, 0:T], in_=cos_t)
        nc.vector.tensor_copy(out=x_sb[:, T:2 * T], in_=sin_t)

        # ------------------- small params -------------------
        w5c_sb = geo.tile([128, OC, NM], cdt, name="w5c_sb", tag="w5c_sb")
        nc.sync.dma_start(out=w5c_sb, in_=w5c_d[:, :, :])
        w5f_sb = geo.tile([128, OC, NM], f32, name="w5f_sb", tag="w5f_sb")
        nc.sync.dma_start(out=w5f_sb, in_=w5f_d[:, :, :])
        bias_sb = geo.tile([128, 4, OC, NM], f32, name="bias_sb", tag="bias_sb")
        nc.sync.dma_start(out=bias_sb, in_=bias_d[:, :, :, :])
        b5v_sb = geo.tile([1, NM], f32, name="b5v_sb", tag="b5v_sb")
        nc.sync.dma_start(out=b5v_sb, in_=b5v_d[:, :])

        s1 = geo.tile([128, T], f32, name="s1", tag="s1")
        s2t = geo.tile([128, T], f32, name="s2t", tag="s2t")
        nc.vector.memset(s1, 0.0)
        nc.vector.memset(s2t, 0.0)

        idN = geo.tile([128, 128], f32, name="idN", tag="idN")
        from concourse.masks import make_identity
        make_identity(nc, idN)

        pe_ps = epool.tile([1, NM], f32, name="pe_ps", tag="pe_ps")

        # ------------------- MLP fwd+bwd (weights as moving operand) -----
        # All matvecs stream the WEIGHT tile as the moving operand (N=HID
        # wide) with the activation chunk stationary. Results land as per-
        # model rows in a shared [NM, 512] PSUM tile (sequential accumulation
        # groups), then one ACT copy + PE transposes ([NM,128] -> [128,NM]
        # against a small identity) return them to column layout.
        GJ = min(8, IC1)        # layer-1 i-chunks per DMA
        BW = min(512, 128 * IC1)  # backward sweep i-block width
        CPB = BW // 128
        NBLK = (128 * IC1) // BW

        # Each model's result row is a [1, 512] PSUM tile in its own bank
        # (PE outputs start at partition 0). The rows are then stacked into
        # one SBUF tile at partitions 32*m (legal compute-engine bases) and
        # PE-transposed [128,128]; valid model columns come out at free-dim
        # columns 32*m (strided AP for consumers). The stack tile is zeroed
        # per round so the transpose only sees finite values.
        def alloc_rowblk():
            return [prowpool.tile([1, 512], f32, name=f"rowm{m}", tag=f"rowm{m}")
                    for m in range(NM)]

        def rows_to_cols(rows, width, consume):
            rowsb = gtmp.tile([128, 512], f32, name="rowsb", tag="rowsb")
            nc.vector.memset(rowsb, 0.0)
            for m in range(NM):
                nc.scalar.copy(rowsb[32 * m:32 * m + 1, 0:width],
                               rows[m][0:1, 0:width])
            for j in range(width // 128):
                ptr = pcpool.tile([128, 128], f32, name="ptr", tag="ptr")
                nc.tensor.transpose(ptr, rowsb[:, j * 128:(j + 1) * 128], idN)
                pcols = ptr.rearrange("p (n q) -> p n q", q=32)[:, :, 0]
                consume(j, pcols[:, 0:NM])

        h_alls = []

        # --- layer 1 forward: z1 rows, model-sequential groups
        z1blk = alloc_rowblk()
        for m in range(NM):
            w1r = w1_d[m].rearrange("(c p) o -> p c o", p=128)
            for j in range(IC1 // GJ):
                w1sb = w1pool.tile([128, GJ, HID], cdt, name="w1sb", tag="w1sb")
                nc.sync.dma_start(out=w1sb, in_=w1r[:, j * GJ:(j + 1) * GJ, :])
                for c in range(GJ):
                    jj = j * GJ + c
                    nc.tensor.matmul(
                        z1blk[m][0:1, 0:HID],
                        x_sb[:, jj:jj + 1],
                        w1sb[:, c, :],
                        start=(jj == 0), stop=(jj == IC1 - 1),
                    )

        def make_h(l, prow):
            h_all = geo.tile([128, OC, NM], cdt, name=f"h{l}", tag=f"h{l}")
            zf = gtmp.tile([128, NM], f32, name="zf", tag="zf")

            def consume(j, pcols):
                vt(zf, pcols, bias_sb[:, l, j, :], ADD)
                nc.vector.tensor_relu(h_all[:, j, :], zf)

            rows_to_cols(prow, HID, consume)
            h_alls.append(h_all)

        make_h(0, z1blk)

        # --- hidden layers forward (model-lockstep per layer)
        for l in range(3):
            whsbs = []
            for m in range(NM):
                whsb = whpool.tile([128, OC, HID], cdt,
                                   name=f"whm{m}", tag=f"whm{m}")
                nc.sync.dma_start(
                    out=whsb, in_=wh_d[l, m].rearrange("(c p) o -> p c o", p=128))
                whsbs.append(whsb)
            zblk = alloc_rowblk()
            for m in range(NM):
                for ic in range(OC):
                    nc.tensor.matmul(
                        zblk[m][0:1, 0:HID],
                        h_alls[l][:, ic, m:m + 1],
                        whsbs[m][:, ic, :],
                        start=(ic == 0), stop=(ic == OC - 1),
                    )
            make_h(l + 1, zblk)

        # --- energy readout + backward seed
        h4 = h_alls[3]
        for m in range(NM):
            for j in range(OC):
                nc.tensor.matmul(
                    pe_ps[0:1, m:m + 1],
                    w5c_sb[:, j, m:m + 1],
                    h4[:, j, m:m + 1],
                    start=(j == 0), stop=(j == OC - 1),
                )
        mask = gtmp.tile([128, OC, NM], f32, name="mask", tag="mask")
        nc.vector.tensor_scalar(mask, h4, 0.0, None, ISGT)
        gz_all = hpool.tile([128, OC, NM], cdt, name="gz_all", tag="gz_all")
        vt(gz_all, w5f_sb, mask, MUL)

        # --- hidden layers backward (lockstep)
        for l in [2, 1, 0]:
            whtsbs = []
            for m in range(NM):
                whtsb = whpool.tile([128, OC, HID], cdt,
                                    name=f"whtm{m}", tag=f"whm{m}")
                nc.sync.dma_start(
                    out=whtsb, in_=wht_d[l, m].rearrange("(c p) i -> p c i", p=128))
                whtsbs.append(whtsb)
            gblk = alloc_rowblk()
            for m in range(NM):
                for o in range(OC):
                    nc.tensor.matmul(
                        gblk[m][0:1, 0:HID],
                        gz_all[:, o, m:m + 1],
                        whtsbs[m][:, o, :],
                        start=(o == 0), stop=(o == OC - 1),
                    )
            mask = gtmp.tile([128, OC, NM], f32, name="mask", tag="mask")
            nc.vector.tensor_scalar(mask, h_alls[l], 0.0, None, ISGT)
            gz_next = hpool.tile([128, OC, NM], cdt, name="gz_all", tag="gz_all")

            def consume_g(j, pcols, gz_next=gz_next, mask=mask):
                vt(gz_next[:, j, :], pcols, mask[:, j, :], MUL)

            rows_to_cols(gblk, HID, consume_g)
            gz_all = gz_next

        # --- layer-1 backward sweep -> gx columns for all models
        gxall = geo.tile([128, IC1, NM], f32, name="gxall", tag="gxall")
        w1trs = [w1t_d[m].rearrange("(c p) i -> p c i", p=128)
                 for m in range(NM)]

        def make_consume_gx(ib):
            def consume_gx(j, pcols):
                nc.vector.tensor_copy(gxall[:, ib * CPB + j, :], pcols)
            return consume_gx

        pending = None
        for ib in range(NBLK):
            w1tblk = w1tpool.tile([128, NM, OC, BW], cdt,
                                  name="w1tblk", tag="w1tblk")
            for m in range(NM):
                for o in range(OC):
                    nc.sync.dma_start(
                        out=w1tblk[:, m, o, :],
                        in_=w1trs[m][:, o, ib * BW:(ib + 1) * BW])
            gxblk = alloc_rowblk()
            for m in range(NM):
                for o in range(OC):
                    nc.tensor.matmul(
                        gxblk[m][0:1, 0:BW],
                        gz_all[:, o, m:m + 1],
                        w1tblk[:, m, o, :],
                        start=(o == 0), stop=(o == OC - 1),
                    )
            # transposes run one block behind the matmuls so PE never waits
            # on the ACT row copies
            if pending is not None:
                p_blk, p_ib = pending
                rows_to_cols(p_blk, BW, make_consume_gx(p_ib))
            pending = (gxblk, ib)
        p_blk, p_ib = pending
        rows_to_cols(p_blk, BW, make_consume_gx(p_ib))

        # --- jac + variance statistics per model
        for m in range(NM):
            jac = gtmp.tile([128, T], f32, name="jac", tag="jac")
            jtmp = gtmp.tile([128, T], f32, name="jtmp", tag="jtmp")
            vt(jac, cos_t, gxall[:, T:2 * T, m], MUL)
            vt(jtmp, sin_t, gxall[:, 0:T, m], MUL)
            vt(jac, jac, jtmp, SUB)
            vt(jac, jac, maskdiv_sb, MUL)      # zero padded CVs
            vt(s1, s1, jac, ADD)
            vt(jtmp, jac, jac, MUL)
            vt(s2t, s2t, jtmp, ADD)

        # ------------------- cross-core reduction -------------------
        el = geo.tile([1, NM], f32, name="el", tag="el")
        vt(el, pe_ps[0:1, :], b5v_sb, ADD)
        ecol = geo.tile([128, 1], f32, name="ecol", tag="ecol")
        nc.vector.memset(ecol, 0.0)
        nc.vector.tensor_reduce(ecol[0:1, 0:1], el, mybir.AxisListType.X, ADD)

        ccin = dram.tile([128, 2 * T + 1], f32, name="ccin")
        cc_addr = "Shared" if cfg.n_cores > 4 else "Local"
        ccout = dram.tile([128, 2 * T + 1], f32, name="ccout", addr_space=cc_addr)
        nc.sync.dma_start(out=ccin[:, 0:T], in_=s1)
        nc.sync.dma_start(out=ccin[:, T:2 * T], in_=s2t)
        nc.sync.dma_start(out=ccin[:, 2 * T:2 * T + 1], in_=ecol)
        nc.gpsimd.collective_compute(
            "AllReduce",
            ADD,
            replica_groups=[list(range(cfg.n_cores))],
            ins=[ccin.opt()],
            outs=[ccout.opt()],
        )
        cc_sb = geo.tile([128, 2 * T + 1], f32, name="cc_sb", tag="cc_sb")
        nc.sync.dma_start(out=cc_sb, in_=ccout[:, :])

        # ------------------- sigma switch + outputs -------------------
        M = float(cfg.m_total)
        gbar = geo.tile([128, T], f32, name="gbar", tag="gbar")
        vs(gbar, cc_sb[:, 0:T], 1.0 / M)
        vvar = geo.tile([128, T], f32, name="vvar", tag="vvar")
        vt(vvar, gbar, gbar, MUL)
        rowsum = geo.tile([128, 1], f32, name="rowsum", tag="rowsum")
        nc.vector.scalar_tensor_tensor(
            vvar, vvar, -M, cc_sb[:, T:2 * T], MUL, ADD,
            accum_out=rowsum)                  # S2 - M*gbar^2 (+ row sums)
        ones_t = geo.tile([128, 1], f32, name="ones_t", tag="ones_t")
        nc.vector.memset(ones_t, 1.0)
        tot_ps = epool.tile([1, 1], f32, name="tot_ps", tag="tot_ps")
        nc.tensor.matmul(tot_ps[0:1, 0:1], ones_t, rowsum, start=True, stop=True)
        md = geo.tile([1, 1], f32, name="md", tag="md")
        nc.scalar.activation(md, tot_ps, mybir.ActivationFunctionType.Sqrt,
                             scale=1.0 / (cfg.c_true * (M - 1.0)))
        # smooth switch: the clamp implements the reference's floor-based
        # cases exactly (isw>=1 -> arg=-pi/2 -> sigma=1; isw<0 -> +pi/2 -> 0)
        den = cfg.e1 - cfg.e0
        arg = geo.tile([1, 1], f32, name="arg", tag="arg")
        pi = float(np.pi)
        vs(arg, md, pi / den, pi / 2 - pi * cfg.e1 / den, MUL, ADD)
        vs(arg, arg, -pi / 2, pi / 2, MAXOP, MINOP)
        sinv = geo.tile([1, 1], f32, name="sinv", tag="sinv")
        nc.scalar.activation(sinv, arg, mybir.ActivationFunctionType.Sin)
        sig = geo.tile([1, 1], f32, name="sig", tag="sig")
        vs(sig, sinv, -0.5, 0.5, MUL, ADD)

        eout = geo.tile([1, 1], f32, name="eout", tag="eout")
        vs(eout, cc_sb[0:1, 2 * T:2 * T + 1], 1.0 / M)
        vt(eout, eout, sig, MUL)
        nc.sync.dma_start(out=energy_d[:, :], in_=eout)

        # broadcast sigma across partitions via DRAM bounce
        sigd = dram.tile([1, 1], f32, name="sigd")
        nc.sync.dma_start(out=sigd[:, :], in_=sig)
        sigb = geo.tile([128, 1], f32, name="sigb", tag="sigb")
        nc.sync.dma_start(out=sigb, in_=sigd.to_broadcast((128, 1)))

        wv = geo.tile([128, T], f32, name="wv", tag="wv")
        nc.vector.tensor_scalar(wv, gbar, sigb, None, MUL)
        wneg = geo.tile([128, T], f32, name="wneg", tag="wneg")
        vs(wneg, wv, -1.0)
        fcon = geo.tile([128, 4, 3, T], f32, name="fcon", tag="fcon")
        for c in range(3):
            vt(fcon[:, 0, c, :], Fc[:, c, :], wv, MUL)
            vt(fcon[:, 1, c, :], g1[:, c, :], wv, MUL)
            vt(fcon[:, 2, c, :], g2[:, c, :], wv, MUL)
            vt(fcon[:, 3, c, :], Hc[:, c, :], wneg, MUL)
        nc.sync.dma_start(out=fcon_d[:, :, :, :], in_=fcon)

    nc.finalize()   # Bacc: runs lowering passes (reg alloc, wait splitting)
    return nc


# ---------------------------------------------------------------------------
# host-side data prep
# ---------------------------------------------------------------------------

def _make_in_maps(inputs, cfg: Cfg):
    npdt = cfg.npdt
    T = cfg.t_cols
    NM = cfg.nm
    OC = cfg.oc
    HID = cfg.hid
    C_PAD = cfg.c_pad
    C_TRUE = cfg.c_true
    IN = cfg.in_dim
    CIN_TRUE = 2 * C_TRUE

    positions = np.asarray(inputs["positions"], np.float32)
    colvar = np.asarray(inputs["colvar_idx"], np.int32)
    box = np.diag(np.asarray(inputs["boxvectors"], np.float32)).copy()

    # padded CV index table (repeat row 0; padded rows are masked out)
    pad = np.zeros((C_PAD, 4), np.int32)
    pad[:C_TRUE] = colvar
    pad[C_TRUE:] = colvar[0]
    flat = pad.reshape(-1)
    sel = positions[flat].reshape(C_PAD, 4, 3)           # host gather
    geom = np.ascontiguousarray(
        sel.reshape(T, 128, 4, 3).transpose(1, 2, 3, 0)).astype(np.float32)

    boxq = np.empty((128, 2, 3, T), np.float32)
    boxq[:, 0] = box[None, :, None]
    boxq[:, 1] = (np.float32(1.0) / box)[None, :, None]

    cv_idx = (np.arange(T)[None, :] * 128 + np.arange(128)[:, None])
    maskdiv = (cv_idx < C_TRUE).astype(np.float32)

    W1 = np.asarray(inputs["W1"], np.float32)
    Whs = [np.asarray(inputs[f"W{i}"], np.float32) for i in (2, 3, 4)]
    W5 = np.asarray(inputs["W5"], np.float32)
    bs = [np.asarray(inputs[f"b{i}"], np.float32) for i in (1, 2, 3, 4)]
    b5 = np.asarray(inputs["b5"], np.float32)

    in_maps = []
    for k in range(cfg.n_cores):
        mods = slice(k * NM, (k + 1) * NM)
        w1p = np.zeros((NM, IN, HID), np.float32)
        w1p[:, :C_TRUE] = W1[mods, :C_TRUE]
        w1p[:, C_PAD:C_PAD + C_TRUE] = W1[mods, C_TRUE:CIN_TRUE]
        w1t = np.ascontiguousarray(w1p.transpose(0, 2, 1))
        wh = np.stack([w[mods] for w in Whs])            # [3, NM, HID, HID]
        wht = np.ascontiguousarray(wh.transpose(0, 1, 3, 2))
        w5c = np.ascontiguousarray(
            W5[mods, :, 0].reshape(NM, OC, 128).transpose(2, 1, 0))  # [128,OC,NM]
        bias = np.ascontiguousarray(
            np.stack([b[mods, 0, :] for b in bs])        # [4l, NM, HID]
            .reshape(4, NM, OC, 128).transpose(3, 0, 2, 1))  # [128,4,OC,NM]
        b5v = b5[mods, 0, 0].reshape(1, NM).astype(np.float32)
        in_maps.append({
            "geom": geom, "boxq": boxq, "maskdiv": maskdiv,
            "w1": w1p.astype(npdt), "w1t": w1t.astype(npdt),
            "wh": wh.astype(npdt), "wht": wht.astype(npdt),
            "w5c": w5c.astype(npdt), "w5f": w5c.astype(np.float32),
            "bias": bias.astype(np.float32), "b5v": b5v,
        })
    return in_maps, (pad, colvar)


def _assemble(results, inputs, cfg: Cfg):
    T = cfg.t_cols
    colvar = np.asarray(inputs["colvar_idx"], np.int32)
    fc = np.asarray(results[0]["fcontrib"], np.float32)   # [128,4,3,T]
    contrib = fc.transpose(3, 0, 1, 2).reshape(cfg.c_pad, 4, 3)[:cfg.c_true]
    forces = np.zeros((np.asarray(inputs["positions"]).shape[0], 3), np.float32)
    np.add.at(forces, colvar.reshape(-1), contrib.reshape(-1, 3))
    energy = np.float32(np.asarray(results[0]["energy"])[0, 0])
    return energy, forces


_NC_CACHE = {}


def _get_nc(cfg: Cfg):
    if cfg not in _NC_CACHE:
        _NC_CACHE[cfg] = _build_nc(cfg)
    return _NC_CACHE[cfg]


def run_raw(inputs, cfg: Cfg = REAL_CFG, trace: bool = False, tmpdir=None):
    """Run on hardware; returns ((energy, forces), BassKernelResults)."""
    nc = _get_nc(cfg)
    in_maps, _ = _make_in_maps(inputs, cfg)
    res = run_bass_kernel_spmd(
        nc, in_maps, core_ids=list(range(cfg.n_cores)), trace=trace,
        tmpdir=tmpdir)
    return _assemble(res.results, inputs, cfg), res


def kernel(**inputs):
    (energy, forces), _ = run_raw(inputs, REAL_CFG, trace=False)
    return energy, forces
